# revision 1
# baseline (speedup 1.0000x reference)
"""Fused multi-head attention block (B=2, N=4096, C=768, H=12, D=64) for 8
Trainium2 NeuronCores.

Sharding: core c -> (batch b = c // 4, head-group g = c % 4, heads
[3g, 3g+1, 3g+2]).  Megatron-style: qkv weights column-split per head
group, proj weights row-split; each core emits a partial [N, C] output
and the host sums the 4 partials per batch and adds proj_b.

Per-core kernel v2 (ACT-floor design):
  - x^T and all weights SBUF-resident in bf16 (no per-chunk DMA).
  - h-major attention per 512-query chunk: S tiles are [128, 3, 512]
    f32 PSUM (3 key-blocks x 512 queries), exp'ed in one ACT
    instruction of free-size 1536 (amortizes the ~480ns/instr ACT
    overhead; ACT is the bottleneck engine at ~456us).
  - exp computed with bias -2 (softmax shift invariance; fp8 P/V was
    tried and rejected: 6%/3% quantization noise exceeds the 2e-2 gate).
  - AV in bf16: P^T written by ACT directly as bf16, V^T transposed
    into 3D Vaug [128, NB, 80] bf16 tiles with a ones-column (col 64)
    for the softmax denominators.
  - Software pipelining: per tile, S(t) is emitted before AV(t-1) so
    the PE never blocks on ACT; normalize and y-proj of chunk Q-1 and
    q-proj of chunk Q+1 are woven into chunk Q's instruction stream to
    keep the PE dense (HAM clock gate stays at 8/8 = 2.4 GHz).
  - Softmax denominators: oacc row 64 -> DVE reciprocal (NOT
    reciprocal_approx_fast: the custom DVE op corrupts SBUF on HW),
    broadcast via PE outer product, normalize via one DVE multiply
    reading PSUM directly.
  - Phase 1 (k/v projection + V transpose) is interleaved with chunk
    0's head-0 attention stream so ACT starts early.
"""

import sys

sys.path.insert(0, "/opt/trn_rl_repo")

from contextlib import ExitStack

import numpy as np
import ml_dtypes

import concourse.bacc as bacc
import concourse.bass as bass
import concourse.mybir as mybir
import concourse.tile as tile

B, N, C, H, D = 2, 4096, 768, 12, 64
SCALE = D ** -0.5
F32 = mybir.dt.float32
F32R = mybir.dt.float32r
BF16 = mybir.dt.bfloat16
FP8 = mybir.dt.float8e4
BF16NP = ml_dtypes.bfloat16

EXPB = -2.0  # exp(S + EXPB): softmax-invariant shift, keeps P < 240 (fp8e4 max)

# column layout of wqkv (output dims of the projection):
# m0 q01 (q_ha|q_hb) 0:128 | m1 k01 128:256 | m2 [q_hc|q_hc] 256:384
# m3 k2 384:448 | m4 v01 448:576 | m5 v2 576:640
MOFF = [0, 128, 256, 384, 448, 576]
MW = [128, 128, 128, 64, 128, 64]

DR = mybir.MatmulPerfMode.DoubleRow


def build_nc(seq=N, debug=False):
    NS = seq // 512   # 512-wide query chunks
    NB = seq // 128   # 128-wide key blocks
    # S-tile schedule: groups of key blocks per exp tile (3,3,...,2)
    TILES = []
    b = 0
    while b < NB:
        n = min(3, NB - b)
        if NB - b == 4:
            n = 2  # avoid a trailing 1-block tile
        TILES.append(list(range(b, b + n)))
        b += n
    NT = len(TILES)

    nc = bacc.Bacc("TRN2", target_bir_lowering=False, debug=False, num_devices=8)
    xt = nc.dram_tensor("xt", [768, seq], BF16, kind="ExternalInput").ap()
    wqkv = nc.dram_tensor("wqkv", [768, 640], BF16, kind="ExternalInput").ap()
    wb = nc.dram_tensor("wb", [128, 6], F32, kind="ExternalInput").ap()
    pwt = nc.dram_tensor("pwt", [384, 768], BF16, kind="ExternalInput").ap()
    ident = nc.dram_tensor("ident", [128, 64], BF16, kind="ExternalInput").ap()
    y = nc.dram_tensor("y", [seq, 768], F32, kind="ExternalOutput").ap()
    if debug:
        dbg = {
            "dq01": nc.dram_tensor("dq01", [128, seq], BF16, kind="ExternalOutput").ap(),
            "dq2": nc.dram_tensor("dq2", [128, seq], BF16, kind="ExternalOutput").ap(),
            "dka": nc.dram_tensor("dka", [128, seq], BF16, kind="ExternalOutput").ap(),
            "dkb": nc.dram_tensor("dkb", [128, seq], BF16, kind="ExternalOutput").ap(),
            "dkc": nc.dram_tensor("dkc", [128, seq], BF16, kind="ExternalOutput").ap(),
            "dva": nc.dram_tensor("dva", [128, (seq // 128) * 80], BF16, kind="ExternalOutput").ap(),
            "dosb": nc.dram_tensor("dosb", [195, 512], F32, kind="ExternalOutput").ap(),
            "dS": nc.dram_tensor("dS", [128, 1536], F32, kind="ExternalOutput").ap(),
            "dP": nc.dram_tensor("dP", [128, 1536], BF16, kind="ExternalOutput").ap(),
            "dotp": nc.dram_tensor("dotp", [384, 512], BF16, kind="ExternalOutput").ap(),
        }

    with tile.TileContext(nc) as tc, ExitStack() as ctx:
        const = ctx.enter_context(tc.tile_pool(name="const", bufs=1))
        big = ctx.enter_context(tc.tile_pool(name="big", bufs=1))
        vst_pool = ctx.enter_context(tc.tile_pool(name="vst", bufs=2))
        pt_pool = ctx.enter_context(tc.tile_pool(name="ptp", bufs=3))
        osb_pool = ctx.enter_context(tc.tile_pool(name="osb", bufs=4))
        rr_pool = ctx.enter_context(tc.tile_pool(name="rrp", bufs=4))
        ysb_pool = ctx.enter_context(tc.tile_pool(name="ysb", bufs=2))
        stp = ctx.enter_context(tc.tile_pool(name="stp", bufs=2, space="PSUM"))
        oaccp = ctx.enter_context(tc.tile_pool(name="oac", bufs=1, space="PSUM"))
        psm = ctx.enter_context(tc.tile_pool(name="psm", bufs=1, space="PSUM"))

        # ---- constants / weights ----
        w_sb = []
        for cch in range(6):
            t = const.tile([128, 640], BF16, tag=f"w{cch}", name=f"w{cch}")
            nc.sync.dma_start(t[:], wqkv[cch * 128:(cch + 1) * 128, :])
            w_sb.append(t)
        wb_sb = const.tile([128, 6], F32, tag="wb")
        nc.sync.dma_start(wb_sb[:], wb[:])
        id_sb = const.tile([128, 64], BF16, tag="id")
        nc.sync.dma_start(id_sb[:], ident[:])
        pw_sb = [const.tile([128, 768], BF16, tag=f"pw{h}", name=f"pwt{h}")
                 for h in range(3)]
        ones_sb = const.tile([128, 64], F32R, tag="ones")
        nc.vector.memset(ones_sb[:].bitcast(F32), 1.0)
        expb_sb = const.tile([128, 1], F32, tag="expb")
        nc.vector.memset(expb_sb[:], EXPB)

        # x^T resident: xst[cch][s] = [128, 512] bf16, DMA'd s-major so
        # phase 1 can start after 6 transfers.
        xst = [[None] * NS for _ in range(6)]
        for s in range(NS):
            for cch in range(6):
                t = const.tile([128, 512], BF16, tag=f"x{cch}_{s}", name="xs")
                nc.sync.dma_start(
                    t[:], xt[cch * 128:(cch + 1) * 128, s * 512:(s + 1) * 512])
                xst[cch][s] = t
        for h in range(3):
            nc.sync.dma_start(pw_sb[h][:], pwt[h * 128:(h + 1) * 128, :])

        # ---- persistent tensors ----
        q01 = big.tile([128, seq], BF16, tag="q01")
        q2 = big.tile([128, seq], BF16, tag="q2")
        ka = big.tile([128, seq], BF16, tag="ka")
        kb = big.tile([128, seq], BF16, tag="kb")
        kc = big.tile([128, seq], BF16, tag="kc")
        nc.gpsimd.memset(ka[64:128, :].bitcast(mybir.dt.uint16), 0)
        nc.gpsimd.memset(kb[0:64, :].bitcast(mybir.dt.uint16), 0)
        nc.gpsimd.memset(kc[64:128, :].bitcast(mybir.dt.uint16), 0)
        # Vaug: [128 keys, NB blocks, 80] fp8; col 64 = 1.0 (denominator row),
        # cols 0:64 = V^T block; cols 65:79 pad (never read).
        vaug = [big.tile([128, NB, 80], BF16, tag=f"va{h}", name=f"va{h}")
                for h in range(3)]
        for h in range(3):
            nc.gpsimd.memset(vaug[h][:], 1.0)
        otp = [big.tile([128, 512], BF16, tag=f"otp{h}", name=f"otp{h}")
               for h in range(3)]
        for h in range(3):
            nc.gpsimd.memset(otp[h][:].bitcast(mybir.dt.uint16), 0)

        heads = [(ka, q01), (kb, q01), (kc, q2)]
        osb_log = []
        dbg_cap = {}
        if debug:
            dbg_cap["S"] = big.tile([128, 1536], F32, tag="dbgS", name="dbgS")

        def copyback(dst, srcap, bias_ap):
            nc.vector.tensor_scalar_add(dst, srcap, bias_ap)

        # ---------- emission helpers ----------

        def emit_qproj(Qn):
            """Project q for chunk Qn into q01/q2 (one stp tile)."""
            ps = stp.tile([128, 3, 512], F32, tag="stp", name="psq")
            for mi, m in enumerate((0, 2)):
                for cch in range(6):
                    nc.tensor.matmul(
                        ps[:, mi:mi + 1, :],
                        lhsT=w_sb[cch][:, MOFF[m]:MOFF[m] + 128],
                        rhs=xst[cch][Qn][:],
                        start=(cch == 0),
                        stop=(cch == 5),
                    )
            qs = slice(Qn * 512, (Qn + 1) * 512)
            copyback(q01[:, qs], ps[:, 0:1, :], wb_sb[:, 0:1])
            copyback(q2[:, qs], ps[:, 1:2, :], wb_sb[:, 2:3])

        def emit_phase1_chunk(s):
            """k/v projection + V transpose for x chunk s (keys 4s..4s+3)."""
            ss = slice(s * 512, (s + 1) * 512)
            psA = stp.tile([128, 3, 512], F32, tag="stp", name="psA")
            for mi, m in enumerate((1, 3, 4)):
                w = MW[m]
                for cch in range(6):
                    nc.tensor.matmul(
                        psA[0:w, mi:mi + 1, :],
                        lhsT=w_sb[cch][:, MOFF[m]:MOFF[m] + w],
                        rhs=xst[cch][s][:],
                        start=(cch == 0),
                        stop=(cch == 5),
                    )
            psB = stp.tile([128, 3, 512], F32, tag="stp", name="psB")
            for cch in range(6):
                nc.tensor.matmul(
                    psB[0:64, 0:1, :],
                    lhsT=w_sb[cch][:, MOFF[5]:MOFF[5] + 64],
                    rhs=xst[cch][s][:],
                    start=(cch == 0),
                    stop=(cch == 5),
                )
            # copybacks (+bias)
            copyback(ka[0:64, ss], psA[0:64, 0:1, :], wb_sb[0:64, 1:2])
            copyback(kb[64:128, ss], psA[64:128, 0:1, :], wb_sb[64:128, 1:2])
            copyback(kc[0:64, ss], psA[0:64, 1:2, :], wb_sb[0:64, 3:4])
            vst01 = vst_pool.tile([128, 512], BF16, tag="vst", name="vst01")
            copyback(vst01[:], psA[:, 2:3, :], wb_sb[:, 4:5])
            vst2 = vst_pool.tile([128, 512], BF16, tag="vst2", name="vst2")
            copyback(vst2[0:64, :], psB[0:64, 0:1, :], wb_sb[0:64, 5:6])
            # V transpose: 4 blocks per head into psB slices, then one
            # strided DVE copy+cast into the fp8 Vaug tile.
            vsrc = [(vst01, 0), (vst01, 64), (vst2, 0)]
            slots = [(1, 0), (1, 256), (2, 0)]
            for h in range(3):
                vs, rb = vsrc[h]
                sl, co = slots[h]
                for j in range(4):
                    nc.tensor.matmul(
                        psB[:, sl:sl + 1, co + j * 64:co + (j + 1) * 64],
                        lhsT=vs[rb:rb + 64, j * 128:(j + 1) * 128],
                        rhs=id_sb[rb:rb + 64, :],
                        start=True,
                        stop=True,
                    )
                nc.vector.tensor_copy(
                    vaug[h][:, 4 * s:4 * s + 4, 0:64],
                    psB[:, sl:sl + 1, co:co + 256],
                )

        def emit_norm(ctx_nh):
            """Normalize O'^T of (chunk, head): otp[h] = osb * (1/sums)."""
            h, osb, rr = ctx_nh
            psb = psm.tile([128, 512], F32, tag="psm", name="psb")
            nc.tensor.matmul(
                psb[0:64, :],
                lhsT=ones_sb[64:65, 0:64],
                rhs=rr[64:65, :],
                start=True,
                stop=True,
            )
            nc.vector.tensor_mul(otp[h][0:64, :], osb[0:64, :], psb[0:64, :])

        def emit_yproj(Q, nt):
            """y rows [Q*512 + nt*128 : +128] from normalized otp."""
            ps = stp.tile([128, 3, 512], F32, tag="stp", name="psy")
            for sl, co, cw in ((0, 0, 512), (1, 512, 256)):
                for h in range(3):
                    nc.tensor.matmul(
                        ps[:, sl:sl + 1, 0:cw],
                        lhsT=otp[h][:, nt * 128:(nt + 1) * 128],
                        rhs=pw_sb[h][:, co:co + cw],
                        start=(h == 0),
                        stop=(h == 2),
                    )
            ysb = ysb_pool.tile([128, 768], F32, tag="ysb", name="ysb")
            nc.vector.tensor_copy(ysb[:, 0:512], ps[:, 0:1, :])
            nc.vector.tensor_copy(ysb[:, 512:768], ps[:, 1:2, 0:256])
            r0 = Q * 512 + nt * 128
            nc.sync.dma_start(y[r0:r0 + 128, :], ysb[:])

        def head_stream(Q, h):
            """Generator: one attention head for one query chunk.

            Yields once per S-tile (a weave point).  PE emission order is
            S(t) then AV(t-1) so the PE never waits on ACT in-order.
            """
            kt, qt = heads[h]
            qs = slice(Q * 512, (Q + 1) * 512)
            oacc = oaccp.tile([65, 512], F32, tag="oacc", name="oacc")
            prevs = []  # up to 2 pending (pt, blocks)
            for t, blocks in enumerate(TILES):
                ps = stp.tile([128, 3, 512], F32, tag="stp", name="ps")
                for j, blk in enumerate(blocks):
                    nc.tensor.matmul(
                        ps[:, j:j + 1, :],
                        lhsT=kt[:, blk * 128:(blk + 1) * 128],
                        rhs=qt[:, qs],
                        start=True,
                        stop=True,
                    )
                nl = len(blocks)
                pt = pt_pool.tile([128, 3, 512], BF16, tag="pt", name="pt")
                nc.scalar.activation(
                    pt[:, 0:nl, :], ps[:, 0:nl, :],
                    mybir.ActivationFunctionType.Exp, bias=expb_sb[:],
                )
                if debug and Q == NS - 1 and h == 0 and t == 0:
                    nc.vector.tensor_copy(dbg_cap["S"][:], ps[:, 0:3, :])
                    nc.sync.dma_start(dbg["dP"][:], pt[:, 0:3, :])
                if len(prevs) == 2:
                    p = prevs.pop(0)
                    emit_av(oacc, h, p, start=(p[1][0] == 0), stop=False)
                prevs.append((pt, blocks))
                yield
            for i, p in enumerate(prevs):
                emit_av(oacc, h, p, start=(p[1][0] == 0),
                        stop=(i == len(prevs) - 1))
            # drain + reciprocal of the denominator row
            osb = osb_pool.tile([65, 512], F32, tag="osb", name="osb")
            osb_log.append(osb)
            nc.vector.tensor_copy(osb[:], oacc[:])
            rr = rr_pool.tile([65, 512], F32, tag="rr", name="rr")
            nc.vector.reciprocal(rr[64:65, :], osb[64:65, :])
            rrr = rr_pool.tile([65, 512], F32R, tag="rrr", name="rrr")
            nc.vector.tensor_copy(rrr[64:65, :], rr[64:65, :])
            yield (h, osb, rrr)

        def emit_av(oacc, h, prev, start, stop):
            pt, blocks = prev
            n = len(blocks)
            for j, blk in enumerate(blocks):
                nc.tensor.matmul(
                    oacc[:],
                    lhsT=vaug[h][:, blk:blk + 1, 0:65],
                    rhs=pt[:, j:j + 1, :],
                    start=(start and j == 0),
                    stop=(stop and j == n - 1),
                )

        # ---------- top-level emission ----------

        emit_qproj(0)

        # Phase 1 interleaved with chunk-0 head-0 attention.
        h0 = head_stream(0, 0)
        t_done = 0  # tiles of (0,0) emitted
        for s in range(NS):
            emit_phase1_chunk(s)
            # blocks 0..4(s+1)-1 now exist; emit available (0,0) tiles
            while t_done < NT and TILES[t_done][-1] < 4 * (s + 1):
                next(h0)
                t_done += 1
        norm_q = []  # pending (h, osb, rr) normalize contexts
        for r in h0:
            if r is not None:
                norm_q.append((0, r))
        # chunk 0 heads 1, 2 (phase 1 done; plain streams)
        for h in (1, 2):
            for r in head_stream(0, h):
                if r is not None:
                    norm_q.append((0, r))
        emit_qproj(1)

        # Steady state: chunk Q attention with woven normalize / y-proj of
        # chunk Q-1 and q-proj of chunk Q+1.  Pending actions are drained
        # one per S-tile (t >= 1) to keep the PE stream dense.
        for Q in range(1, NS):
            actions = [lambda r=r: emit_norm(r[1]) for r in norm_q]
            actions += [lambda Qp=Q - 1, nt=nt: emit_yproj(Qp, nt)
                        for nt in range(4)]
            if Q + 1 < NS:
                actions.append(lambda Qn=Q + 1: emit_qproj(Qn))
            norm_q = []
            for h in range(3):
                t = 0
                for r in head_stream(Q, h):
                    if r is not None:
                        norm_q.append((Q, r))
                        break
                    if t >= 1 and actions:
                        actions.pop(0)()
                    t += 1
            for a in actions:  # drain any leftovers (small-NT configs)
                a()

        # tail: normalize + y-proj of the last chunk
        for _, r in norm_q:
            emit_norm(r)
        for nt in range(4):
            emit_yproj(NS - 1, nt)

        if debug:
            nc.sync.dma_start(dbg["dS"][:], dbg_cap["S"][:])
            nc.sync.dma_start(dbg["dq01"][:], q01[:])
            nc.sync.dma_start(dbg["dq2"][:], q2[:])
            nc.sync.dma_start(dbg["dka"][:], ka[:])
            nc.sync.dma_start(dbg["dkb"][:], kb[:])
            nc.sync.dma_start(dbg["dkc"][:], kc[:])
            nc.sync.dma_start(dbg["dva"][:], vaug[0][:])
            for j in range(3):
                nc.sync.dma_start(
                    dbg["dosb"][j * 65:(j + 1) * 65, :], osb_log[-3 + j][:])
            for hh in range(3):
                nc.sync.dma_start(
                    dbg["dotp"][hh * 128:(hh + 1) * 128, :], otp[hh][:])

    nc.compile()
    return nc


def host_prep(x, qkv_w, qkv_b, proj_w, seq=N):
    """Build the 8 per-core input maps."""
    f = np.float32
    x = np.asarray(x, f)
    qkv_w = np.asarray(qkv_w, f)
    qkv_b = np.asarray(qkv_b, f)
    proj_w = np.asarray(proj_w, f)

    xts = [np.ascontiguousarray(x[b].T).astype(BF16NP) for b in range(B)]
    id2 = np.concatenate([np.eye(64, dtype=f)] * 2, axis=0).astype(BF16NP)

    in_maps = []
    for core in range(8):
        b, g = core // 4, core % 4
        ha, hb_, hc = 3 * g, 3 * g + 1, 3 * g + 2

        def Wrow(base, h):
            return qkv_w[base + h * 64: base + (h + 1) * 64, :]  # [64, 768]

        def brow(base, h):
            return qkv_b[base + h * 64: base + (h + 1) * 64]

        cols = np.concatenate(
            [
                Wrow(0, ha).T * SCALE, Wrow(0, hb_).T * SCALE,   # q01
                Wrow(C, ha).T, Wrow(C, hb_).T,                   # k01 -> ka/kb
                Wrow(0, hc).T * SCALE, Wrow(0, hc).T * SCALE,    # q2 duplicated
                Wrow(C, hc).T,                                   # k2
                Wrow(2 * C, ha).T, Wrow(2 * C, hb_).T,           # v01
                Wrow(2 * C, hc).T,                               # v2
            ],
            axis=1,
        )  # [768, 640]
        bias = np.concatenate(
            [
                brow(0, ha) * SCALE, brow(0, hb_) * SCALE,
                brow(C, ha), brow(C, hb_),
                brow(0, hc) * SCALE, brow(0, hc) * SCALE,
                brow(C, hc),
                brow(2 * C, ha), brow(2 * C, hb_), brow(2 * C, hc),
            ]
        )  # [640]
        wbm = np.zeros((128, 6), f)
        for m in range(6):
            wbm[0:MW[m], m] = bias[MOFF[m]:MOFF[m] + MW[m]]
        pwt = np.zeros((384, 768), f)
        for i, h in enumerate((ha, hb_, hc)):
            pwt[i * 128:i * 128 + 64, :] = proj_w.T[h * 64:(h + 1) * 64, :]

        in_maps.append(
            {
                "xt": xts[b][:, :seq],
                "wqkv": np.ascontiguousarray(cols).astype(BF16NP),
                "wb": wbm,
                "pwt": pwt.astype(BF16NP),
                "ident": id2,
            }
        )
    return in_maps


_nc_cache = {}


def _get_nc(seq=N):
    key = (seq,)
    if key not in _nc_cache:
        _nc_cache[key] = build_nc(seq)
    return _nc_cache[key]


def kernel(x, qkv_w, qkv_b, proj_w, proj_b, _trace=False):
    from concourse.bass_utils import run_bass_kernel_spmd

    nc = _get_nc()
    in_maps = host_prep(x, qkv_w, qkv_b, proj_w)
    res = run_bass_kernel_spmd(nc, in_maps, list(range(8)), trace=_trace)
    proj_b = np.asarray(proj_b, np.float32)
    out = np.zeros((B, N, C), np.float32)
    for b in range(B):
        acc = np.zeros((N, C), np.float32)
        for g in range(4):
            acc += res.results[b * 4 + g]["y"]
        out[b] = acc + proj_b[None, :]
    if _trace:
        return out, res
    return out

